# revision 1
# baseline (speedup 1.0000x reference)
"""Trainium2 Bass kernel: GAT-style message passing layer (2 edge sets) + GRUCell + LayerNorm.

Sharding: target nodes i across 8 cores (256 per core). Each core gets
adj[:, shard], w[:, shard] (cast fp16 on host) and a replicated copy of x / params.
All math happens on device; host only slices / transposes / casts / replicates.

Score pipeline per (j-tile, set, head), fully fused in ONE custom DVE op:
    wc = w + adj                       (gpsimd tensor add; edge iff wc >= 1)
    u  = select(wc < 1, -FLT_MAX, leaky_relu((B + a_nb)*(wc-1), 0.2))
    e  = exp(u)                        (ACT; masked lanes underflow to exactly 0)
No softmax max-subtraction is needed: scores are O(+-8) so exp is safe, and the
reference's max-subtraction cancels in the normalization.

Aggregation: U^T[(d|denom), i] = msg_ext^T @ e on PE, with a ones column in
msg_ext producing the softmax denominator for free. Normalize by 1/denom
(fast reciprocal + ones-broadcast matmul), GRU via host-pretransposed weights,
LayerNorm with Newton-iteration rsqrt (keeps a single ACT table set: exp+tanh).
"""

import numpy as np

import concourse.bass as bass
import concourse.mybir as mybir
from concourse import bacc
import concourse.tile as tile
from concourse.bass_utils import run_bass_kernel_spmd

# problem constants (hardcoded; harness provides full inputs)
N, D, DH, H = 2048, 256, 256, 4
DHEAD = DH // H
NCORES = 8
S = N // NCORES          # 256 targets per core
JT = N // 128            # 16 j-tiles
KT = D // 128            # 2 k-tiles over d
F16 = mybir.dt.float16
F32 = mybir.dt.float32
AF = mybir.ActivationFunctionType
ALU = mybir.AluOpType

LAST_EXEC_NS = None

# ---------------------------------------------------------------- custom DVE op
_GAT_OP = None


def _register_gat_score():
    """u = select(wc < 1, -FLT_MAX, leaky_relu((B + nb) * (wc - 1), alpha))
    in0 = wc (w+adj), in1 = B (a_cur+ba row-bcast), s0 = nb col [P,1], s1 = 1.0,
    imm2 = alpha."""
    global _GAT_OP
    if _GAT_OP is not None:
        return _GAT_OP
    import concourse.dve_ops as dve_ops
    from concourse.dve_spec import (
        C0, C1, C2, One, Spec, Src0, Src1, _has_src1, lower as spec_lower,
        maxx, select,
    )
    from concourse.dve_uop import DveOpSpec

    name = "GAT_SCORE_ANT"
    for op in dve_ops.OPS:
        if op.name == name:
            _GAT_OP = op
            return op

    _s = Src1 + C0
    _w = Src0 - One
    _q = _s * _w
    body = select(Src0 < One, C1, maxx(_q, _q * C2))

    def _ref(in0, in1, s0, s1, imm2):
        q = (in1.astype(np.float32) + s0) * (in0.astype(np.float32) - 1.0)
        lr = np.maximum(q, q * np.float32(imm2))
        return np.where(in0.astype(np.float32) < 1.0, np.float32(s1), lr).astype(
            np.float32
        )

    spec = Spec(body=body, reference=_ref)
    row = dve_ops._CUSTOM_DVE_ROW_BASE + len(dve_ops.OPS)
    shas = {}
    for ver in ("v3", "v4"):
        try:
            uops = spec_lower(spec, ver=ver)
            shas[ver] = DveOpSpec(
                name=name, opcode=row, uops=uops, rd1_en=_has_src1(spec)
            ).sha(ver)
        except Exception:
            pass
    op = dve_ops.DveOp(name, spec, subdim=False, uops_sha=shas,
                       perf_en={"v3": True, "v4": True})
    dve_ops.OPS.append(op)
    dve_ops.CUSTOM_DVE_SPECS[name] = spec
    dve_ops._SUB_OPCODE_FOR_NAME[name] = row
    _GAT_OP = op
    return op


# ---------------------------------------------------------------- bass program
_NC_CACHE = None


def _build_nc(dbg=False):
    global _NC_CACHE
    if _NC_CACHE is not None:
        return _NC_CACHE
    gat = _register_gat_score()

    nc = bacc.Bacc("TRN2", target_bir_lowering=False, debug=False,
                   enable_asserts=False)

    def din(nm, shape, dt):
        return nc.dram_tensor(nm, list(shape), dt, kind="ExternalInput").ap()

    # big streamed shards, tile-packed on host: [128, JT*256]
    wp = [din(f"wp{e}", (128, JT * S), F16) for e in range(2)]
    ap_ = [din(f"ap{e}", (128, JT * S), F16) for e in range(2)]
    xT_d = din("xT", (D, N), F16)            # x transposed, fp16
    xisl_d = din("xisl", (D, S), F16)        # x^T columns of this core's shard
    wiT_d = din("wiT", (2 * DH, 3 * D), F16)  # gru_wih^T  [512, 768]
    whT_d = din("whT", (D, 3 * D), F16)       # gru_whh^T  [256, 768]
    WmT_d = din("WmT", (2 * D, DH), F16)      # [Wm0^T; Wm1^T]  [512, 256]
    WaTnb_d = din("WaTnb", (D, 8), F16)       # nb halves of Wa0/Wa1, transposed
    WaTcur_d = din("WaTcur", (D, 8), F16)
    ba_row_d = din("ba_row", (1, 8), F16)     # [ba0(4), ba1(4)]
    selB_d = din("selB", (8, 8 * 128), F16)   # one-hot row selectors for B bcast
    bihr_d = din("bihr", (1, 3 * D), F16)
    bhhr_d = din("bhhr", (1, 3 * D), F16)
    bmc_d = din("bmc", (64, 8), F16)          # bm_cat split in 8 x 64 pieces
    lnG_d = din("lnG", (128, D), F32)         # ln_g broadcast to 128 partitions
    lnB_d = din("lnB", (128, D), F32)
    ones_d = din("ones", (1, 128), F16)
    ident_d = din("ident", (128, 128), F16)
    onecol_d = din("onecol", (128, 4), F16)
    onesf_d = din("onesf", (1, 64), F32)

    out_d = nc.dram_tensor("out", [S, D], F32, kind="ExternalOutput").ap()
    dbg_d = {}
    if dbg:
        for nm, shape in [("d_anb0", (128, 8)), ("d_B0", (128, 1024)),
                          ("d_u0", (128, 1024)), ("d_et0", (128, 1024)),
                          ("d_me0", (128, 264)), ("d_U00", (65, 256)),
                          ("d_U01", (65, 256)), ("d_U02", (65, 256)),
                          ("d_U03", (65, 256)), ("d_et1", (128, 1024)),
                          ("d_et15", (128, 1024)), ("d_me15", (128, 264)),
                          ("d_msgT0", (64, 256)), ("d_aTc", (8, 256)),
                          ("d_gh0", (128, 768)), ("d_gi0", (128, 768)),
                          ("d_hh0", (128, 256))]:
            dbg_d[nm] = nc.dram_tensor(nm, list(shape), F32,
                                       kind="ExternalOutput").ap()

    def ddump(nm, ap):
        if not dbg or nm not in dbg_d:
            return
        t = cp.tile(list(dbg_d[nm].shape), F32, tag=nm, name=nm)
        nc.vector.tensor_copy(t[:], ap)
        nc.sync.dma_start(out=dbg_d[nm][:, :], in_=t[:])

    with tile.TileContext(nc) as tc:
        with (
            tc.tile_pool(name="const", bufs=1) as cp,
            tc.tile_pool(name="stream", bufs=2) as sp,
            tc.tile_pool(name="work", bufs=3) as wkp,
            tc.tile_pool(name="msg", bufs=1) as mp,
        ):
            # ---------------- constants into SBUF
            def load(pool, nm, src, shape, dt, tag=None):
                t = pool.tile(shape, dt, tag=tag or nm, name=tag or nm)
                nc.sync.dma_start(out=t[:], in_=src)
                return t

            xT = [load(cp, f"xT{k}", xT_d[128 * k:128 * (k + 1), :],
                       [128, N], F16) for k in range(KT)]
            xisl = [load(cp, f"xisl{k}", xisl_d[128 * k:128 * (k + 1), :],
                        [128, S], F16) for k in range(KT)]
            wiT = [load(cp, f"wiT{p}", wiT_d[64 * p:64 * (p + 1), :],
                        [64, 3 * D], F16) for p in range(8)]
            whT = [load(cp, f"whT{k}", whT_d[128 * k:128 * (k + 1), :],
                        [128, 3 * D], F16) for k in range(KT)]
            WmT = [load(cp, f"WmT{k}", WmT_d[128 * k:128 * (k + 1), :],
                        [128, DH], F16) for k in range(2 * KT)]
            WaTnb = [load(cp, f"WaTnb{k}", WaTnb_d[128 * k:128 * (k + 1), :],
                          [128, 8], F16) for k in range(KT)]
            WaTcur = [load(cp, f"WaTcur{k}", WaTcur_d[128 * k:128 * (k + 1), :],
                           [128, 8], F16) for k in range(KT)]
            ba_row = load(cp, "ba_row", ba_row_d[:, :], [1, 8], F16)
            selB = load(cp, "selB", selB_d[:, :], [8, 8 * 128], F16)
            bihr = load(cp, "bihr", bihr_d[:, :], [1, 3 * D], F16)
            bhhr = load(cp, "bhhr", bhhr_d[:, :], [1, 3 * D], F16)
            bmc = load(cp, "bmc", bmc_d[:, :], [64, 8], F16)
            lnG = load(cp, "lnG", lnG_d[:, :], [128, D], F32)
            lnB = load(cp, "lnB", lnB_d[:, :], [128, D], F32)
            ones = load(cp, "ones", ones_d[:, :], [1, 128], F16)
            onesf = load(cp, "onesf", onesf_d[:, :], [1, 64], F32)

            ident = load(cp, "ident", ident_d[:, :], [128, 128], F16)

            me_t = [cp.tile([128, 4, 66], F16, tag=f"me{t}", name=f"me{t}")
                    for t in range(JT)]
            for t in range(JT):
                nc.sync.dma_start(out=me_t[t][:, :, DHEAD:65], in_=onecol_d[:, :])

            psA_cm = tc.tile_pool(name="psA", bufs=2, space="PSUM")
            psU_cm = tc.tile_pool(name="psU", bufs=4, space="PSUM")
            psB_cm = tc.tile_pool(name="psB", bufs=1, space="PSUM")
            psA = psA_cm.__enter__(); psU = psU_cm.__enter__(); psB = psB_cm.__enter__()

            # ---------------- a = x @ Wa^T pieces (for all j), + ba on nb half
            anb = []
            for t in range(JT):
                ps = psA.tile([128, 8], F32, tag="ps_m", name="ps_a")
                for k in range(KT):
                    nc.tensor.matmul(ps[:], xT[k][:, 128 * t:128 * (t + 1)],
                                     WaTnb[k][:], start=(k == 0), stop=False)
                nc.tensor.matmul(ps[:], ones[:], ba_row[:], start=False,
                                 stop=True)
                a = cp.tile([128, 8], F32, tag=f"anb{t}", name=f"anb{t}")
                nc.vector.tensor_copy(a[:], ps[:])
                if t == 0:
                    ddump("d_anb0", a[:])
                anb.append(a)

            # a_cur on this core's i-shard, transposed to rows -> aTc [8, 256]
            aTc = cp.tile([8, S], F16, tag="aTc", name="aTc")
            for ih in range(2):
                ps = psA.tile([128, 8], F32, tag="ps_m", name="ps_a")
                for k in range(KT):
                    nc.tensor.matmul(ps[:], xisl[k][:, 128 * ih:128 * (ih + 1)],
                                     WaTcur[k][:], start=(k == 0),
                                     stop=(k == KT - 1))
                ac = wkp.tile([128, 8], F16, tag="acur", name="acur")
                nc.vector.tensor_copy(ac[:], ps[:])
                pst = psB.tile([8, 128], F16, tag="misc", name="ps_at")
                nc.tensor.transpose(pst[:], ac[:], ident[:])
                nc.vector.tensor_copy(aTc[:, 128 * ih:128 * (ih + 1)], pst[:])

            ddump("d_aTc", aTc[:])

            # x islice natural layout [i, d] fp16 (for the GRU h-mix)
            xn = []
            for ih in range(2):
                t = cp.tile([128, D], F16, tag=f"xn{ih}", name=f"xn{ih}")
                for k in range(KT):
                    pst = psB.tile([128, 128], F16, tag="misc", name="ps_xt")
                    nc.tensor.transpose(
                        pst[:], xisl[k][:, 128 * ih:128 * (ih + 1)], ident[:])
                    nc.vector.tensor_copy(t[:, 128 * k:128 * (k + 1)], pst[:])
                xn.append(t)

            # ---------------- per edge-set: B, scores, exp, msg, aggregation
            msgT = []   # 8 pieces [64, 256] fp16: c-block (e*4+h)
            for e in range(2):
                # B[p, (h,i)] = a_cur[i0+i, h(set e)]  via one-hot selector matmuls
                psb = psB.tile([128, 4 * S], F32, tag="misc", name="ps_B")
                for h in range(4):
                    k = 4 * e + h
                    nc.tensor.matmul(psb[:, S * h:S * (h + 1)],
                                     selB[:, 128 * k:128 * (k + 1)], aTc[:],
                                     start=True, stop=True)
                B = sp.tile([128, 4 * S], F16, tag="Bsb", name="Bsb")
                nc.vector.tensor_copy(B[:], psb[:])
                if e == 0:
                    ddump("d_B0", B[:])

                # stream shard tiles
                wsb = sp.tile([128, JT * S], F16, tag="wsb", name="wsb")
                asb = sp.tile([128, JT * S], F16, tag="asb", name="asb")
                nc.sync.dma_start(out=wsb[:], in_=wp[e][:, :])
                nc.sync.dma_start(out=asb[:], in_=ap_[e][:, :])

                U = [psU.tile([65, S], F32, tag="ps_U", name="ps_U") for _ in range(4)]
                for t in range(JT):
                    sl = slice(S * t, S * (t + 1))
                    nc.gpsimd.tensor_tensor(wsb[:, sl], wsb[:, sl], asb[:, sl],
                                            ALU.add)
                    # fused scores -> fp16 u (2 j-tiles batched per exp)
                    if t % 2 == 0:
                        u2 = wkp.tile([128, 2, 4 * S], F16, tag="u", name="u", bufs=2)
                    for h in range(4):
                        nc.vector._custom_dve(
                            gat,
                            out=u2[:, t % 2, S * h:S * (h + 1)],
                            in0=wsb[:, sl],
                            in1=B[:, S * h:S * (h + 1)],
                            s0=anb[t][:, 4 * e + h:4 * e + h + 1],
                            s1=-200.0,
                            imm2=0.2,
                        )
                    if e == 0 and t == 0:
                        ddump("d_u0", u2[:, 0, :])
                    if t % 2 == 1:
                        et2 = wkp.tile([128, 2, 4 * S], F16, tag="et", name="et", bufs=2)
                        nc.scalar.activation(et2[:, :, :], u2[:, :, :], AF.Exp)
                    if e == 0 and t == 1:
                        ddump("d_et0", et2[:, 0, :])
                        ddump("d_et1", et2[:, 1, :])
                    if e == 0 and t == 15:
                        ddump("d_et15", et2[:, 1, :])

                    # msg tile for this j-tile: [128, 4, 65] (64 msg + ones col)
                    psm = psA.tile([128, 4, DHEAD], F32, tag="ps_m", name="ps_m")
                    for k in range(KT):
                        nc.tensor.matmul(psm[:], xT[k][:, 128 * t:128 * (t + 1)],
                                         WmT[2 * e + k][:], start=(k == 0),
                                         stop=(k == KT - 1))
                    me = me_t[t]
                    if t % 2 == 0:
                        nc.vector.tensor_copy(me[:, :, 0:DHEAD], psm[:])
                    else:
                        nc.scalar.copy(me[:, :, 0:DHEAD], psm[:])

                    if e == 0 and t == 0:
                        ddump("d_me0", me[:, :, :].rearrange("p a b -> p (a b)"))
                    if e == 0 and t == 15:
                        ddump("d_me15", me[:, :, :].rearrange("p a b -> p (a b)"))
                    if t % 2 == 1:
                        for tt in (t - 1, t):
                            for h in range(4):
                                nc.tensor.matmul(U[h][:], me_t[tt][:, h, 0:65],
                                                 et2[:, tt % 2, S * h:S * (h + 1)],
                                                 start=(tt == 0), stop=(tt == JT - 1))

                # normalize: msgT_piece = U[0:64] / denom(U[64])
                if e == 0:
                    ddump("d_U00", U[0][:, :])
                    ddump("d_U01", U[1][:, :])
                    ddump("d_U02", U[2][:, :])
                    ddump("d_U03", U[3][:, :])
                for h in range(4):
                    rawU = wkp.tile([65, S], F32, tag="rawU", name="rawU")
                    nc.scalar.copy(rawU[:], U[h][:, :])
                    dz = wkp.tile([1, S], F32, tag="dz", name="dz")
                    nc.sync.dma_start(out=dz[0:1, :], in_=rawU[64:65, :])
                    rd = wkp.tile([1, S], F32, tag="rd", name="rd")
                    nc.vector.reciprocal_approx_fast(rd[0:1, :], dz[0:1, :])
                    rb = psA.tile([64, S], F32, tag="ps_m", name="ps_rb")
                    nc.tensor.matmul(rb[:], onesf[:], rd[0:1, :], start=True,
                                     stop=True)
                    piece = mp.tile([64, S], F16, tag=f"msgT{4 * e + h}", name=f"msgT{4 * e + h}")
                    nc.vector.tensor_tensor(piece[:], rawU[0:64, :], rb[:], ALU.mult)
                    if e == 0 and h == 0:
                        ddump("d_msgT0", piece[:])
                    msgT.append(piece)

            psB_cm.__exit__(None, None, None)
            psU_cm.__exit__(None, None, None)
            psA_cm.__exit__(None, None, None)
            psG_cm = tc.tile_pool(name="psG", bufs=1, space="PSUM")
            psG = psG_cm.__enter__()

            # ---------------- GRU input bias row: bih + bm_cat @ wih^T
            psbias = psG.tile([1, 3 * D], F32, tag="ps_bias", name="ps_bias")
            for ns, (lo, hi) in enumerate(((0, 512), (512, 768))):
                for p in range(8):
                    nc.tensor.matmul(psbias[:, lo:hi], bmc[:, p:p + 1],
                                     wiT[p][:, lo:hi], start=(p == 0),
                                     stop=False)
                nc.tensor.matmul(psbias[:, lo:hi], ones[0:1, 0:1],
                                 bihr[:, lo:hi], start=False, stop=True)
            biasr = cp.tile([1, 3 * D], F16, tag="biasr", name="biasr")
            nc.vector.tensor_copy(biasr[:], psbias[:])

            # ---------------- GRU + LayerNorm per i-half
            for ih in range(2):
                ihs = slice(128 * ih, 128 * (ih + 1))
                # gh = x @ whh^T + bhh
                psgh = psG.tile([128, 3 * D], F32, tag="ps_gh", name="ps_gh")
                for lo, hi in ((0, 512), (512, 768)):
                    for k in range(KT):
                        nc.tensor.matmul(psgh[:, lo:hi], xisl[k][:, ihs],
                                         whT[k][:, lo:hi], start=(k == 0),
                                         stop=False)
                    nc.tensor.matmul(psgh[:, lo:hi], ones[:], bhhr[:, lo:hi],
                                     start=False, stop=True)
                gh = wkp.tile([128, 3 * D], F32, tag="gh", name="gh")
                nc.vector.tensor_copy(gh[:], psgh[:])
                if ih == 0:
                    ddump("d_gh0", gh[:])

                # gi = msgcat @ wih^T + (bih + bm@wihT)
                psgi = psG.tile([128, 3 * D], F32, tag="ps_gi", name="ps_gi")
                for lo, hi in ((0, 512), (512, 768)):
                    for p in range(8):
                        nc.tensor.matmul(psgi[:, lo:hi], msgT[p][:, ihs],
                                         wiT[p][:, lo:hi], start=(p == 0),
                                         stop=False)
                    nc.tensor.matmul(psgi[:, lo:hi], ones[:], biasr[:, lo:hi],
                                     start=False, stop=True)

                if ih == 0:
                    ddump("d_gi0", psgi[:, :])
                # r/z = sigmoid(gi+gh) = 0.5*tanh(0.5*(gi+gh)) + 0.5 ; n = tanh
                trz = wkp.tile([128, 2 * D], F32, tag="trz", name="trz")
                nc.vector.tensor_tensor(trz[:], psgi[:, 0:2 * D], gh[:, 0:2 * D],
                                        ALU.add)
                th = wkp.tile([128, 2 * D], F32, tag="th", name="th")
                nc.scalar.activation(th[:], trz[:], AF.Tanh, scale=0.5)
                rz = wkp.tile([128, 2 * D], F32, tag="rz", name="rz")
                nc.vector.tensor_scalar(rz[:], th[:], 0.5, 0.5, ALU.mult,
                                        ALU.add)
                t1 = wkp.tile([128, D], F32, tag="t1", name="t1")
                nc.vector.tensor_tensor(t1[:], rz[:, 0:D], gh[:, 2 * D:3 * D],
                                        ALU.mult)
                t2 = wkp.tile([128, D], F32, tag="t2", name="t2")
                nc.vector.tensor_tensor(t2[:], t1[:], psgi[:, 2 * D:3 * D],
                                        ALU.add)
                nn_ = wkp.tile([128, D], F32, tag="nn", name="nn")
                nc.scalar.activation(nn_[:], t2[:], AF.Tanh)
                # h = n + z*(x - n)
                t3 = wkp.tile([128, D], F32, tag="t3", name="t3")
                nc.vector.tensor_tensor(t3[:], xn[ih][:], nn_[:], ALU.subtract)
                t4 = wkp.tile([128, D], F32, tag="t4", name="t4")
                nc.vector.tensor_tensor(t4[:], t3[:], rz[:, D:2 * D], ALU.mult)
                hh = wkp.tile([128, D], F32, tag="hh", name="hh")
                nc.vector.tensor_tensor(hh[:], nn_[:], t4[:], ALU.add)

                if ih == 0:
                    ddump("d_hh0", hh[:])
                # LayerNorm
                st = wkp.tile([128, 6], F32, tag="st", name="st")
                nc.vector.bn_stats(out=st[:], in_=hh[:])
                mv = wkp.tile([128, 2], F32, tag="mv", name="mv")
                nc.vector.bn_aggr(out=mv[:], in_=st[:])
                veps = wkp.tile([128, 1], F32, tag="veps", name="veps")
                nc.vector.tensor_scalar_add(veps[:], mv[:, 1:2], 1e-5)
                rv = wkp.tile([128, 1], F32, tag="rv", name="rv")
                nc.vector.reciprocal(rv[:], veps[:])
                # Newton rsqrt: y0 = 0.5 + 0.5/v ; y <- y*(1.5 - 0.5*v*y^2)
                y = wkp.tile([128, 1], F32, tag="y", name="y")
                nc.vector.tensor_scalar(y[:], rv[:], 0.5, 0.5, ALU.mult, ALU.add)
                for _ in range(5):
                    q1 = wkp.tile([128, 1], F32, tag="q1", name="q1")
                    nc.vector.tensor_tensor(q1[:], y[:], y[:], ALU.mult)
                    q2 = wkp.tile([128, 1], F32, tag="q2", name="q2")
                    nc.vector.tensor_tensor(q2[:], q1[:], veps[:], ALU.mult)
                    q3 = wkp.tile([128, 1], F32, tag="q3", name="q3")
                    nc.vector.tensor_scalar(q3[:], q2[:], -0.5, 1.5, ALU.mult,
                                            ALU.add)
                    y2 = wkp.tile([128, 1], F32, tag="y", name="y")
                    nc.vector.tensor_tensor(y2[:], y[:], q3[:], ALU.mult)
                    y = y2
                hn = wkp.tile([128, D], F32, tag="hn", name="hn")
                nc.vector.tensor_scalar(hn[:], hh[:], mv[:, 0:1], y[:],
                                        ALU.subtract, ALU.mult)
                og = wkp.tile([128, D], F32, tag="og", name="og")
                nc.vector.tensor_tensor(og[:], hn[:], lnG[:], ALU.mult)
                ob = wkp.tile([128, D], F32, tag="ob", name="ob")
                nc.vector.tensor_tensor(ob[:], og[:], lnB[:], ALU.add)
                nc.sync.dma_start(out=out_d[ihs, :], in_=ob[:])
            psG_cm.__exit__(None, None, None)

    nc.compile()
    _NC_CACHE = nc
    return nc


# ---------------------------------------------------------------- host wrapper
def _sat16(a):
    """f32 -> fp16 with values < 1.0 kept strictly below 1.0 (saturating cast
    so the wc >= 1 edge test survives rounding)."""
    f = a.astype(np.float16)
    f[(f >= 1.0) & (a < 1.0)] = np.float16(1.0 - 2.0 ** -11)
    return f


def _pack_tiles(a):
    """[N, S] -> [128, JT*S]: row-tile t, partition p holds a[t*128+p, :] at
    cols [t*S:(t+1)*S]."""
    n, s = a.shape
    t = n // 128
    return np.ascontiguousarray(
        a.reshape(t, 128, s).transpose(1, 0, 2).reshape(128, t * s))


def kernel(_dbg=False, **inputs):
    global LAST_EXEC_NS
    f16 = np.float16
    x = np.asarray(inputs["axiom_states"], np.float32)
    adj = [np.asarray(inputs["adj0"], np.float32),
           np.asarray(inputs["adj1"], np.float32)]
    w = [np.asarray(inputs["w0"], np.float32),
         np.asarray(inputs["w1"], np.float32)]
    Wm = [np.asarray(inputs["Wm0"], np.float32),
          np.asarray(inputs["Wm1"], np.float32)]
    bm = [np.asarray(inputs["bm0"], np.float32),
          np.asarray(inputs["bm1"], np.float32)]
    Wa = [np.asarray(inputs["Wa0"], np.float32),
          np.asarray(inputs["Wa1"], np.float32)]
    ba = [np.asarray(inputs["ba0"], np.float32),
          np.asarray(inputs["ba1"], np.float32)]
    wih = np.asarray(inputs["gru_wih"], np.float32)
    whh = np.asarray(inputs["gru_whh"], np.float32)
    bih = np.asarray(inputs["gru_bih"], np.float32)
    bhh = np.asarray(inputs["gru_bhh"], np.float32)
    ln_g = np.asarray(inputs["ln_g"], np.float32)
    ln_b = np.asarray(inputs["ln_b"], np.float32)

    xT = np.ascontiguousarray(x.T).astype(f16)                     # [256, 2048]
    wiT = np.ascontiguousarray(wih.T).astype(f16)                  # [512, 768]
    whT = np.ascontiguousarray(whh.T).astype(f16)                  # [256, 768]
    WmT = np.concatenate([Wm[0].T, Wm[1].T], 0).astype(f16)        # [512, 256]
    WaTnb = np.concatenate([Wa[0][:, D:].T, Wa[1][:, D:].T], 1).astype(f16)
    WaTcur = np.concatenate([Wa[0][:, :D].T, Wa[1][:, :D].T], 1).astype(f16)
    ba_row = np.concatenate([ba[0], ba[1]]).reshape(1, 8).astype(f16)
    selB = np.zeros((8, 8 * 128), np.float32)
    for k in range(8):
        selB[k, 128 * k:128 * (k + 1)] = 1.0
    selB = selB.astype(f16)
    bihr = bih.reshape(1, -1).astype(f16)
    bhhr = bhh.reshape(1, -1).astype(f16)
    bm_cat = np.concatenate([bm[0], bm[1]])                        # [512]
    bmc = np.ascontiguousarray(bm_cat.reshape(8, 64).T).astype(f16)  # [64, 8]
    lnG = np.broadcast_to(ln_g, (128, D)).astype(np.float32).copy()
    lnB = np.broadcast_to(ln_b, (128, D)).astype(np.float32).copy()
    ones = np.ones((1, 128), f16)
    onesf = np.ones((1, 64), np.float32)

    nc = _build_nc(dbg=_dbg)

    in_maps = []
    for c in range(NCORES):
        isl = slice(c * S, (c + 1) * S)
        m = {
            "wp0": _pack_tiles(_sat16(w[0][:, isl])),
            "wp1": _pack_tiles(_sat16(w[1][:, isl])),
            "ap0": _pack_tiles(adj[0][:, isl].astype(f16)),
            "ap1": _pack_tiles(adj[1][:, isl].astype(f16)),
            "xT": xT,
            "xisl": np.ascontiguousarray(xT[:, isl]),
            "wiT": wiT, "whT": whT, "WmT": WmT,
            "WaTnb": WaTnb, "WaTcur": WaTcur, "ba_row": ba_row,
            "selB": selB, "bihr": bihr, "bhhr": bhhr, "bmc": bmc,
            "lnG": lnG, "lnB": lnB, "ones": ones, "onesf": onesf,
            "ident": np.eye(128, dtype=f16),
            "onecol": np.ones((128, 4), f16),
        }
        in_maps.append(m)

    import os
    trace = bool(int(os.environ.get("KERNEL_TRACE", "0")))
    if trace:
        import axon_ntff_shim  # noqa: F401  (registers the NTFF hook)
    res = run_bass_kernel_spmd(nc, in_maps, core_ids=list(range(NCORES)),
                               trace=trace)
    LAST_EXEC_NS = res.exec_time_ns
    out = np.concatenate([r["out"] for r in res.results], axis=0)
    if _dbg:
        global LAST_DBG
        LAST_DBG = res.results
    return out

